# revision 1
# baseline (speedup 1.0000x reference)
"""GCN layer v4: balanced node->block packing, Tq=4, self-loops as edges.

Same math as v2 (emb = postscale(dinv_dst) * [(gathered dinv_src*x) @ OH] @ W
+ b), but dst nodes are assigned to blocks by a balancing packer so every
(block, src-chunk) bucket holds <= 512 edges (Tq=4), cutting slot padding
from ~26% to ~6%. Self-loops ride the same slot machinery (w-slot 1.0), so
the separate xself/identity path disappears. The node->slot permutation is
undone on the host after download.
"""

import numpy as np
import ml_dtypes

import concourse.bass as bass
import concourse.tile as tile
from concourse import bacc, mybir
from concourse.bass_utils import run_bass_kernel_spmd

P = 128
F = 128
NC = 8
N = 100000
BLOCKS_PER_CORE = 112
NCHUNK = 4
GRP = 8

BF16 = mybir.dt.bfloat16
F32 = mybir.dt.float32
I16 = mybir.dt.int16

_cache: dict = {}


def _pack_bins(vec, n_bins, cap, bin_cap=P):
    """Exponential-potential vector packing (balances every chunk dim and
    the bin count against their running targets). Returns (bin_of, loads)."""
    n_nodes, k = vec.shape
    tau, cnt_tau = 8.0, 2.0
    mean = vec.sum() / (n_bins * k)
    exp_cnt = n_nodes / n_bins
    vecf = vec.astype(np.float64)
    loads = np.zeros((n_bins, k))
    counts = np.zeros(n_bins)
    bin_of = np.full(n_nodes, -1, dtype=np.int64)
    order = np.argsort(-vec.sum(axis=1), kind="stable")
    tot = float(vec.sum())
    placed = 0.0
    for v in order:
        t = placed / tot
        cand = loads + vecf[v]
        score = np.exp((cand - t * mean) / tau).sum(axis=1) \
            + np.exp((counts + 1 - t * exp_cnt) / cnt_tau)
        score[counts >= bin_cap] = np.inf
        b = int(np.argmin(score))
        bin_of[v] = b
        loads[b] += vecf[v]
        counts[b] += 1
        placed += vecf[v].sum()
    return bin_of, loads.astype(np.int64)


def _host_prep(x, W, b, edge_index, edge_weight, n_nodes, blocks_per_core,
               n_cores, n_chunks=NCHUNK):
    p = P
    npc = blocks_per_core * p
    n_pad = n_cores * npc
    cs = n_pad // n_chunks
    assert cs < 32768
    n_blocks = n_cores * blocks_per_core

    src0 = edge_index[0].astype(np.int64)
    dst0 = edge_index[1].astype(np.int64)
    w0 = edge_weight.astype(np.float64)

    deg = np.bincount(dst0, weights=w0, minlength=n_nodes) + 1.0
    dinv = 1.0 / np.sqrt(deg)

    # self-loops as edges (weight-slot 1.0)
    loop = np.arange(n_nodes, dtype=np.int64)
    src = np.concatenate([src0, loop])
    dst = np.concatenate([dst0, loop])
    w = np.concatenate([w0, np.ones(n_nodes)])

    # per-dst-node chunk-degree vectors (incl self edge); windows hold
    # npq real nodes each so bucket loads are chunk-balanced
    npq = n_nodes // n_chunks
    chunk = (src // npq).astype(np.int64)
    vec = np.zeros((n_nodes, n_chunks), dtype=np.int32)
    np.add.at(vec, (dst, chunk), 1)

    bin_of, loads = _pack_bins(vec, n_blocks, cap=4 * p)
    maxload = loads.max()
    # lanes within each bin (order of assignment)
    order_v = np.argsort(bin_of, kind="stable")
    lane_of = np.zeros(n_nodes, dtype=np.int64)
    binc = np.bincount(bin_of, minlength=n_blocks)
    st = np.zeros(n_blocks, dtype=np.int64)
    st[1:] = np.cumsum(binc)[:-1]
    lane_of[order_v] = np.arange(n_nodes) - st[bin_of[order_v]]
    assert lane_of.max() < p
    row_of = bin_of.astype(np.int64) * p + lane_of   # device row per node

    blk = bin_of[dst].astype(np.int64)
    seg = blk * n_chunks + chunk
    order = np.lexsort((src, seg))
    seg_s = seg[order]
    n_segs = n_blocks * n_chunks
    cnt = np.bincount(seg_s, minlength=n_segs)
    Tq = max(1, int(np.ceil(cnt.max() / p)))
    Sq = Tq * p
    til_e = n_chunks * Tq

    starts = np.zeros(n_segs, dtype=np.int64)
    starts[1:] = np.cumsum(cnt)[:-1]
    pos = np.arange(len(order)) - starts[seg_s]
    slot = seg_s * Sq + pos

    idx_slots = np.zeros(n_segs * Sq, dtype=np.int16)
    idx_slots[slot] = (src[order] - chunk[order] * npq).astype(np.int16)

    q_of = slot // Sq % n_chunks
    pos_in_seg = slot % Sq
    u_of = q_of * Tq + pos_in_seg // p
    lane_s = pos_in_seg % p
    rows = (slot // (Sq * n_chunks)) * (til_e * p) + u_of * p + lane_s
    oh = np.zeros((n_blocks * til_e * p, p), dtype=ml_dtypes.bfloat16)
    oh[rows, lane_of[dst[order]]] = w[order].astype(ml_dtypes.bfloat16)
    oh4 = oh.reshape(n_blocks, til_e, p, p)

    x_pad = np.zeros((n_pad, F), dtype=ml_dtypes.bfloat16)
    xs = (x.astype(np.float64) * dinv[:, None]).astype(ml_dtypes.bfloat16)
    for q in range(n_chunks):
        x_pad[q * cs:q * cs + npq] = xs[q * npq:(q + 1) * npq]

    dinv_dev = np.ones(n_pad)
    dinv_dev[row_of] = dinv
    dinv_blk = dinv_dev.reshape(n_blocks, p)

    w_bf = np.ascontiguousarray(W.astype(ml_dtypes.bfloat16))
    b_f32 = np.ascontiguousarray(b.astype(np.float32).reshape(1, F))

    grp = GRP if blocks_per_core % GRP == 0 else 1
    n_grp = blocks_per_core // grp
    cols_pc = grp * Sq // 16
    n_calls = n_grp * n_chunks

    idx_seg = idx_slots.reshape(n_blocks, n_chunks, Sq)

    in_maps = []
    for c in range(n_cores):
        b0 = c * blocks_per_core
        cb = idx_seg[b0:b0 + blocks_per_core].reshape(n_grp, grp, n_chunks, Sq)
        calls = cb.transpose(0, 2, 1, 3).reshape(n_grp, n_chunks, grp * Sq)
        wrapped = calls.reshape(n_grp, n_chunks, grp * Sq // 16, 16)
        wrapped = wrapped.transpose(0, 1, 3, 2).reshape(n_grp * n_chunks * 16,
                                                        cols_pc)
        idx16 = wrapped.reshape(n_calls, 16, cols_pc).transpose(1, 0, 2)
        idx16 = np.ascontiguousarray(
            np.tile(idx16.reshape(16, n_calls * cols_pc), (8, 1)))

        oh_core = np.ascontiguousarray(
            oh4[b0:b0 + blocks_per_core].transpose(2, 0, 1, 3)
            .reshape(p, blocks_per_core * til_e * p))

        in_maps.append({
            "x": x_pad,
            "w_in": w_bf,
            "b_in": b_f32,
            "idx_in": idx16,
            "oh_in": oh_core,
            "dinv_in": np.ascontiguousarray(
                dinv_blk[b0:b0 + blocks_per_core].T.astype(np.float32)),
            "rdinv_in": np.ascontiguousarray(
                (1.0 / dinv_blk[b0:b0 + blocks_per_core])
                .reshape(1, npc).astype(np.float32)),
        })
    return in_maps, Tq, row_of


def _build_program(Tq, n_pad, blocks_per_core, n_chunks):
    p = P
    npc = blocks_per_core * p
    til_e = n_chunks * Tq
    Sq = Tq * p
    grp = GRP if blocks_per_core % GRP == 0 else 1
    n_grp = blocks_per_core // grp
    cols_pc = grp * Sq // 16
    n_calls = n_grp * n_chunks
    gbufs = 3 if Tq <= 4 else 2

    nc = bacc.Bacc("TRN2", target_bir_lowering=False, debug=False,
                   enable_asserts=False, num_devices=NC,
                   num_swdge_queues=4)

    x_d = nc.dram_tensor("x", [n_pad, F], BF16, kind="ExternalInput")
    w_d = nc.dram_tensor("w_in", [F, F], BF16, kind="ExternalInput")
    b_d = nc.dram_tensor("b_in", [1, F], F32, kind="ExternalInput")
    idx_d = nc.dram_tensor("idx_in", [p, n_calls * cols_pc], I16,
                           kind="ExternalInput")
    oh_d = nc.dram_tensor("oh_in", [p, blocks_per_core * til_e * p], BF16,
                          kind="ExternalInput")
    dinv_d = nc.dram_tensor("dinv_in", [p, blocks_per_core], F32,
                            kind="ExternalInput")
    rdinv_d = nc.dram_tensor("rdinv_in", [1, npc], F32, kind="ExternalInput")
    emb_d = nc.dram_tensor("emb_out", [npc, F], BF16, kind="ExternalOutput")
    relu_d = nc.dram_tensor("relu_out", [npc, F], BF16, kind="ExternalOutput")

    emb_v = emb_d.ap().rearrange("(B q) f -> q B f", q=p)
    relu_v = relu_d.ap().rearrange("(B q) f -> q B f", q=p)

    with tile.TileContext(nc) as tc:
        with (
            tc.tile_pool(name="const", bufs=1) as const_pool,
            tc.tile_pool(name="gather", bufs=gbufs) as gpool,
            tc.tile_pool(name="ohbuf", bufs=4) as ohpool,
            tc.tile_pool(name="aggsb", bufs=3) as aggpool,
            tc.tile_pool(name="outsb", bufs=2) as outpool,
            tc.tile_pool(name="psum_agg", bufs=2, space="PSUM") as ps_agg,
            tc.tile_pool(name="psum_emb", bufs=2, space="PSUM") as ps_emb,
        ):
            w_sb = const_pool.tile([F, F], BF16)
            nc.sync.dma_start(out=w_sb[:], in_=w_d.ap())
            b_sb = const_pool.tile([1, F], F32)
            nc.sync.dma_start(out=b_sb[:], in_=b_d.ap())
            dinv_sb = const_pool.tile([p, blocks_per_core], F32)
            nc.sync.dma_start(out=dinv_sb[:], in_=dinv_d.ap())
            rdinv_sb = const_pool.tile([1, npc], F32)
            nc.sync.dma_start(out=rdinv_sb[:], in_=rdinv_d.ap())
            idx_sb = const_pool.tile([p, n_calls * cols_pc], I16)
            nc.sync.dma_start(out=idx_sb[:], in_=idx_d.ap())

            oh_v = oh_d.ap()

            for g in range(n_grp):
                gq = []
                for q in range(n_chunks):
                    gt = gpool.tile([p, grp * Sq], BF16, tag=f"g{q}")
                    nc.gpsimd.dma_gather(
                        out_ap=gt[:].rearrange("q (j f) -> q j f", f=F),
                        in_ap=x_d.ap()[q * (n_pad // n_chunks):
                                       (q + 1) * (n_pad // n_chunks), :],
                        idxs_ap=idx_sb[:, (g * n_chunks + q) * cols_pc:
                                       (g * n_chunks + q + 1) * cols_pc],
                        num_idxs=grp * Sq,
                        num_idxs_reg=grp * Sq,
                        elem_size=F,
                        single_packet=False,
                        queue_num=(g * n_chunks + q) % 4)
                    gq.append(gt)

                emb_st = outpool.tile([p, grp * F], BF16, tag="emb_st")
                relu_st = outpool.tile([p, grp * F], BF16, tag="relu_st")
                for bi in range(grp):
                    blk = g * grp + bi
                    oh_b = ohpool.tile([p, til_e * p], BF16, tag="oh")
                    nc.scalar.dma_start(
                        out=oh_b[:],
                        in_=oh_v[:, blk * til_e * p:(blk + 1) * til_e * p])
                    agg_ps = ps_agg.tile([p, p], F32)
                    for u in range(til_e):
                        q, t = divmod(u, Tq)
                        nc.tensor.matmul(
                            out=agg_ps[:],
                            lhsT=gq[q][:, (bi * Tq + t) * F:
                                       (bi * Tq + t + 1) * F],
                            rhs=oh_b[:, u * p:(u + 1) * p],
                            start=(u == 0), stop=(u == til_e - 1))

                    agg_sb = aggpool.tile([p, p], BF16)
                    nc.scalar.activation(
                        out=agg_sb[:], in_=agg_ps[:],
                        func=mybir.ActivationFunctionType.Copy)
                    emb_ps = ps_emb.tile([p, F], F32)
                    nc.tensor.matmul(out=emb_ps[:],
                                     lhsT=rdinv_sb[:, blk * p:(blk + 1) * p],
                                     rhs=b_sb[:], start=True, stop=False)
                    nc.tensor.matmul(out=emb_ps[:], lhsT=agg_sb[:],
                                     rhs=w_sb[:], start=False, stop=True)
                    nc.scalar.activation(
                        out=emb_st[:, bi * F:(bi + 1) * F], in_=emb_ps[:],
                        func=mybir.ActivationFunctionType.Copy,
                        scale=dinv_sb[:, blk:blk + 1])
                    nc.scalar.activation(
                        out=relu_st[:, bi * F:(bi + 1) * F], in_=emb_ps[:],
                        func=mybir.ActivationFunctionType.Relu,
                        scale=dinv_sb[:, blk:blk + 1])
                nc.sync.dma_start(
                    out=emb_v[:, g * grp:(g + 1) * grp, :],
                    in_=emb_st[:].rearrange("q (B f) -> q B f", f=F))
                nc.sync.dma_start(
                    out=relu_v[:, g * grp:(g + 1) * grp, :],
                    in_=relu_st[:].rearrange("q (B f) -> q B f", f=F))

    nc.compile()
    return nc


def _get_program(Tq, n_pad, blocks_per_core, n_chunks):
    key = (Tq, n_pad, blocks_per_core, n_chunks)
    if key not in _cache:
        _cache[key] = _build_program(Tq, n_pad, blocks_per_core, n_chunks)
    return _cache[key]


def run(x, W, b, edge_index, edge_weight, n_nodes, blocks_per_core, n_cores,
        n_chunks=NCHUNK, trace=False):
    in_maps, Tq, row_of = _host_prep(x, W, b, edge_index, edge_weight,
                                     n_nodes, blocks_per_core, n_cores,
                                     n_chunks)
    n_pad = n_cores * blocks_per_core * P
    nc = _get_program(Tq, n_pad, blocks_per_core, n_chunks)
    res = run_bass_kernel_spmd(nc, in_maps, list(range(n_cores)), trace=trace)
    emb_dev = np.concatenate([res.results[c]["emb_out"]
                              for c in range(n_cores)], axis=0)
    relu_dev = np.concatenate([res.results[c]["relu_out"]
                               for c in range(n_cores)], axis=0)
    emb = np.asarray(emb_dev)[row_of].astype(np.float32)
    relu = np.asarray(relu_dev)[row_of].astype(np.float32)
    return (emb, relu), res


def kernel(x, W, b, level, edge_index, edge_weight):
    x = np.asarray(x)
    W = np.asarray(W)
    b = np.asarray(b)
    edge_index = np.asarray(edge_index)
    edge_weight = np.asarray(edge_weight)
    (emb, relu), _ = run(x, W, b, edge_index, edge_weight,
                         N, BLOCKS_PER_CORE, NC)
    return emb, relu

